# revision 8
# baseline (speedup 1.0000x reference)
"""Trainium2 Bass kernel for nn_MemAggregator (gnn_message_passing).

Data-parallel over nodes across 8 NeuronCores. Per core (1024 nodes):
  - D = total_node + total_rel built directly by DMA (accum_op=add folds the
    big tensor add into the DMA engines, no vector-engine pass).
  - F = D - e (with the head_rel broadcast folded in via 17 per-node offset
    vectors), squared on the scalar engine, reduced over R on the vector
    engine -> diff scores [nodes, 152].
  - Top-8 per node via the DVE Max8/MaxIndex instructions.
  - The reference's stride-C flat indexing reads rows total_node[8n+t] which
    live in nodes ~n/19; each core materializes that small source window
    (57 nodes) to DRAM and gathers rows with an indirect DMA.
"""

import sys

if "/opt/trn_rl_repo" not in sys.path:
    sys.path.insert(0, "/opt/trn_rl_repo")

import numpy as np

import concourse.bacc as bacc
import concourse.mybir as mybir
import concourse.tile as tile
from concourse.alu_op_type import AluOpType
from concourse.bass import IndirectOffsetOnAxis

N, D, C, R = 8192, 16, 8, 64
NCORES = 8
NLOC = N // NCORES            # 1024 nodes per core
L = C + D * (C + 1)           # 152 candidate rows per node
P = 128                       # partitions = nodes per tile
NT = NLOC // P                # 8 tiles per core
NSRC = 57                     # source-window nodes for the gather
NROWS = NSRC * L              # 8664 rows in the gather table

f32 = mybir.dt.float32
u32 = mybir.dt.uint32
SQUARE = mybir.ActivationFunctionType.Square
AXX = mybir.AxisListType.X


def build_program():
    nc = bacc.Bacc("TRN2", target_bir_lowering=False, debug=False)

    def din(name, shape, dt=f32):
        return nc.dram_tensor(name, shape, dt, kind="ExternalInput").ap()

    def dout(name, shape, dt=f32):
        return nc.dram_tensor(name, shape, dt, kind="ExternalOutput").ap()

    msg_d = din("msg", [NLOC, D, R])
    alpha_d = din("alpha", [NLOC, D])
    ce_d = din("ce", [NLOC, R])
    cnm_d = din("cnm", [NLOC, C, R])
    crm_d = din("crm", [NLOC, C, R])
    nn_d = din("nei_node", [NLOC, D, C, R])
    nr_d = din("nei_rel", [NLOC, D, C, R])
    he_d = din("head_emb", [NLOC, D, R])
    hr_d = din("head_rel", [NLOC, D, R])
    gb_d = din("gbase", [NLOC, 1], u32)
    s_cnm_d = din("s_cnm", [NSRC, C, R])
    s_crm_d = din("s_crm", [NSRC, C, R])
    s_nn_d = din("s_nei_node", [NSRC, D, C, R])
    s_nr_d = din("s_nei_rel", [NSRC, D, C, R])
    s_he_d = din("s_head_emb", [NSRC, D, R])
    s_hr_d = din("s_head_rel", [NSRC, D, R])

    oemb_d = dout("new_emb", [NLOC, R])
    onode_d = dout("new_node", [NLOC, C, R])
    orel_d = dout("new_rel", [NLOC, C, R])

    # Interleaved gather table: row q = [total_node[q] ; total_rel[q]] so one
    # indirect-DMA index fetches both 64-f32 rows.  The HW SWDGE indirect DMA
    # only honors one index per partition ([P,1] offset APs), derives the
    # per-index stride from the innermost source dim, and wants dest tiles at
    # offset 0 — hence a 2D [NROWS, 128] table, 8 gathers per 128-node tile,
    # each into its own full [P, 128] tile.
    ts_both = nc.dram_tensor("ts_both", [NROWS, 2 * R], f32)

    with tile.TileContext(nc) as tc:
        with (
            tc.tile_pool(name="big", bufs=2) as big,
            tc.tile_pool(name="med", bufs=2) as med,
            tc.tile_pool(name="small", bufs=3) as small,
        ):
            # ---- Phase A: materialize the gather source window in DRAM ----
            # total_node rows: l<8 -> cnm[c]; l=8+9d+c (c<8) -> nei_node[d,c];
            # l=8+9d+8 -> head_emb[d].  total_rel analogous, with head_rel
            # broadcast-added onto the nei block.
            ts_n = big.tile([NSRC, L, R], f32, tag="bigtile")
            ts_r = big.tile([NSRC, L, R], f32, tag="bigtile")
            hrs = med.tile([NSRC, D, R], f32, tag="hr")
            nc.sync.dma_start(out=ts_n[:, 0:C, :], in_=s_cnm_d)
            vn = ts_n[:, C:L, :].rearrange("p (d c) r -> p d c r", c=C + 1)
            nc.sync.dma_start(out=vn[:, :, 0:C, :], in_=s_nn_d)
            nc.sync.dma_start(out=vn[:, :, C, :], in_=s_he_d)
            nc.sync.dma_start(out=ts_r[:, 0:C, :], in_=s_crm_d)
            vr = ts_r[:, C:L, :].rearrange("p (d c) r -> p d c r", c=C + 1)
            nc.sync.dma_start(out=vr[:, :, 0:C, :], in_=s_nr_d)
            nc.sync.dma_start(out=vr[:, :, C, :], in_=s_hr_d)
            nc.sync.dma_start(out=hrs[:], in_=s_hr_d)
            nc.vector.tensor_add(
                vr[:, :, 0:C, :],
                vr[:, :, 0:C, :],
                hrs[:].unsqueeze(2).to_broadcast([NSRC, D, C, R]),
            )
            vboth = ts_both.ap().rearrange("(n l) (two r) -> n l two r", l=L, two=2)
            nc.sync.dma_start(out=vboth[:, :, 0, :], in_=ts_n[:])
            nc.sync.dma_start(out=vboth[:, :, 1, :], in_=ts_r[:])

            # ---- Phase B: main loop over node tiles ----
            for t in range(NT):
                s0, s1 = t * P, (t + 1) * P

                Dt = big.tile([P, L, R], f32, tag="bigtile")
                # total_node part (plain writes)
                nc.sync.dma_start(out=Dt[:, 0:C, :], in_=cnm_d[s0:s1])
                v = Dt[:, C:L, :].rearrange("p (d c) r -> p d c r", c=C + 1)
                nc.sync.dma_start(out=v[:, :, 0:C, :], in_=nn_d[s0:s1])
                nc.sync.dma_start(out=v[:, :, C, :], in_=he_d[s0:s1])
                # total_rel part folded in by the DMA engines (CCE add)
                nc.gpsimd.dma_start(
                    out=Dt[:, 0:C, :], in_=crm_d[s0:s1], accum_op=AluOpType.add
                )
                nc.gpsimd.dma_start(
                    out=v[:, :, 0:C, :], in_=nr_d[s0:s1], accum_op=AluOpType.add
                )
                nc.gpsimd.dma_start(
                    out=v[:, :, C, :], in_=hr_d[s0:s1], accum_op=AluOpType.add
                )

                msg_t = med.tile([P, D, R], f32, tag="msg")
                hr_t = med.tile([P, D, R], f32, tag="hr")
                alpha_t = small.tile([P, D], f32, tag="alpha")
                ce_t = small.tile([P, R], f32, tag="ce")
                gb_t = small.tile([P, 1], u32, tag="gb")
                nc.sync.dma_start(out=msg_t[:], in_=msg_d[s0:s1])
                nc.sync.dma_start(out=hr_t[:], in_=hr_d[s0:s1])
                nc.sync.dma_start(out=alpha_t[:], in_=alpha_d[s0:s1])
                nc.sync.dma_start(out=ce_t[:], in_=ce_d[s0:s1])
                nc.sync.dma_start(out=gb_t[:], in_=gb_d[s0:s1])

                # new_emb e = einsum(alpha, msg) + ce
                nm = med.tile([P, D, R], f32, tag="nm")
                nc.vector.tensor_mul(
                    nm[:], msg_t[:], alpha_t[:].unsqueeze(2).to_broadcast([P, D, R])
                )
                e_t = small.tile([P, R], f32, tag="e")
                nc.vector.tensor_reduce(
                    out=e_t[:], in_=nm[:].transpose([0, 2, 1]), axis=AXX,
                    op=AluOpType.add,
                )
                nc.vector.tensor_add(e_t[:], e_t[:], ce_t[:])
                nc.sync.dma_start(out=oemb_d[s0:s1], in_=e_t[:])

                # per-node offset vectors: en = -e ; Ep[d] = head_rel[d] - e
                en_t = small.tile([P, R], f32, tag="en")
                nc.scalar.mul(en_t[:], e_t[:], -1.0)
                ep_t = med.tile([P, D, R], f32, tag="ep")
                nc.vector.tensor_sub(
                    ep_t[:], hr_t[:], e_t[:].unsqueeze(1).to_broadcast([P, D, R])
                )

                # F = D - e (+ head_rel on the nei block, already via Ep)
                nc.vector.tensor_add(
                    Dt[:, 0:C, :],
                    Dt[:, 0:C, :],
                    en_t[:].unsqueeze(1).to_broadcast([P, C, R]),
                )
                nc.vector.tensor_add(
                    v[:, :, 0:C, :],
                    v[:, :, 0:C, :],
                    ep_t[:].unsqueeze(2).to_broadcast([P, D, C, R]),
                )
                nc.vector.tensor_add(
                    v[:, :, C, :],
                    v[:, :, C, :],
                    en_t[:].unsqueeze(1).to_broadcast([P, D, R]),
                )

                # scores = sum_r F^2
                nc.scalar.activation(out=Dt[:], in_=Dt[:], func=SQUARE)
                scores = small.tile([P, L], f32, tag="scores")
                nc.vector.tensor_reduce(
                    out=scores[:], in_=Dt[:], axis=AXX, op=AluOpType.add
                )

                # top-8 (descending) + flat gather indices
                tv = small.tile([P, 8], f32, tag="tv")
                ti = small.tile([P, 8], u32, tag="ti")
                nc.vector.max(out=tv[:], in_=scores[:])
                nc.vector.max_index(out=ti[:], in_max=tv[:], in_values=scores[:])
                gidx = small.tile([P, 8], u32, tag="gidx")
                nc.vector.tensor_add(gidx[:], ti[:], gb_t[:].to_broadcast([P, 8]))

                for j in range(C):
                    gt = small.tile([P, 2 * R], f32, tag=f"gt{j}")
                    nc.gpsimd.indirect_dma_start(
                        out=gt[:], out_offset=None, in_=ts_both.ap(),
                        in_offset=IndirectOffsetOnAxis(ap=gidx[:, j:j + 1], axis=0),
                    )
                    nc.sync.dma_start(out=onode_d[s0:s1, j, :], in_=gt[:, 0:R])
                    nc.sync.dma_start(out=orel_d[s0:s1, j, :], in_=gt[:, R:2 * R])

    nc.compile()
    return nc


def shard_inputs(inputs):
    """Full-problem inputs -> per-core in_maps."""
    gi = {k: np.ascontiguousarray(np.asarray(v, dtype=np.float32))
          for k, v in inputs.items()}
    ce = gi["curr_emb"][:, 0, :]
    alpha = gi["alpha"][:, :, 0]
    cnm = gi["curr_node_mem"][:, 0]
    crm = gi["curr_rel_mem"][:, 0]
    nn_ = gi["nei_node_mem"]
    nr_ = gi["nei_rel_mem"]
    he = gi["head_emb"]
    hr = gi["head_rel_emb"]
    msg = gi["msg"]

    in_maps = []
    for k in range(NCORES):
        s = slice(k * NLOC, (k + 1) * NLOC)
        base = 8 * NLOC * k
        n0 = base // L
        off = base - L * n0
        ss = slice(n0, n0 + NSRC)
        gb = (off + 8 * np.arange(NLOC, dtype=np.uint32)).astype(np.uint32)
        in_maps.append({
            "msg": msg[s], "alpha": np.ascontiguousarray(alpha[s]),
            "ce": np.ascontiguousarray(ce[s]),
            "cnm": np.ascontiguousarray(cnm[s]),
            "crm": np.ascontiguousarray(crm[s]),
            "nei_node": nn_[s], "nei_rel": nr_[s],
            "head_emb": he[s], "head_rel": hr[s],
            "gbase": gb.reshape(NLOC, 1),
            "s_cnm": np.ascontiguousarray(cnm[ss]),
            "s_crm": np.ascontiguousarray(crm[ss]),
            "s_nei_node": np.ascontiguousarray(nn_[ss]),
            "s_nei_rel": np.ascontiguousarray(nr_[ss]),
            "s_head_emb": np.ascontiguousarray(he[ss]),
            "s_head_rel": np.ascontiguousarray(hr[ss]),
        })
    return in_maps


_CACHE = {}


def kernel(**inputs):
    if "nc" not in _CACHE:
        _CACHE["nc"] = build_program()
    nc = _CACHE["nc"]
    in_maps = shard_inputs(inputs)

    from concourse.bass_utils import run_bass_kernel_spmd

    res = run_bass_kernel_spmd(nc, in_maps, list(range(NCORES))).results
    new_emb = np.concatenate([r["new_emb"] for r in res], axis=0)
    new_node = np.concatenate([r["new_node"] for r in res], axis=0)
    new_rel = np.concatenate([r["new_rel"] for r in res], axis=0)
    return new_emb, new_node, new_rel
